# revision 15
# baseline (speedup 1.0000x reference)
"""Trainium2 Bass kernel for nn_DAWN_41549513621652.

Strategy (8 NeuronCores, single chip, no cross-core collectives):
  Dense matmul work (attention, Wo, memory WV, lm_head) runs on device;
  sequential/tiny glue (layernorm, the 512-step SSM scan, routing softmax,
  the rank-128 compress projections h=xn@sc / Q=xn@sc, and the neuron-pool
  contractions nw@{comp,EQ,EK,EV}) runs on host between launches; host also
  sums the 8 per-core Wo partials.

  5 device launches per call:
    A (x2): circuit module, head-sharded — core c owns heads {2c, 2c+1} for
            both batch elements. fp16 is used ONLY for the Q/K score path
            (attention scores are ~1e-4, softmax er-uniform, insensitive);
            the V path / attention output / Wo stay f32r and partials are
            written fp32: the downstream memory module's top-16 selection
            has 16/17-gaps down to 7e-9 and amplifies any upstream drift
            >1e-6 into >2e-2 logit errors (measured).  The softmax
            denominator Z is produced by an extra ones-column appended to V
            in the same accumulating matmul as the attention output.
    C (x2): memory module, token-sharded — core c owns 128 tokens.  Exact
            top-16 in 3 DVE passes (max8 / match_replace / max8, then
            select s >= 16th value), masked exp + Z fused in one
            tensor_tensor_reduce, PE-transpose to nk-major, dense WV
            matmul.  Scores strictly fp32 both layers; layer 1's WV/kV are
            fp16 (they feed only the lm_head, no selection downstream).
    D (x1): lm_head, vocab-sharded — core c owns a 4000-wide slice of the
            32000 vocab, entirely fp16 (halves the 36MB DMA, keeps the PE
            at its ~107us roofline).
"""

import numpy as np

import concourse.bass as bass
import concourse.bacc as bacc
import concourse.mybir as mybir
import concourse.tile as tile
from concourse.bass_utils import run_bass_kernel_spmd
from concourse.masks import make_identity

F32 = mybir.dt.float32
F32R = mybir.dt.float32r
F16 = mybir.dt.float16

# model dims (hardcoded per problem spec)
L, D, H, R, NC, NK, KK, SD, V, B, S = 2, 1024, 16, 128, 64, 1024, 16, 64, 32000, 2, 512
DH = D // H          # 64
T = B * S            # 1024
N_CORES = 8
VSL = V // N_CORES   # 4000 per-core vocab slice
VC = 500             # vocab chunk (psum tile width)
NVC = VSL // VC      # 8
DT = D // 128        # 8 d-tiles
NT = NK // 128       # 8 knowledge tiles
NEG = -1e30
EXPF = mybir.ActivationFunctionType.Exp
COPYF = mybir.ActivationFunctionType.Copy


# ---------------------------------------------------------------- device programs


def _build_A():
    """Circuit module, head-sharded. Per-core inputs:
      h16/h32 [128(R), T]  compressed tokens (host: xn@sc), R-major
      eqs/eks [128(R), B, 128] f16, evs f32  expansion slices (2 heads)
      woT  [128, D]  o_w.T rows for this core's d_in slice
      tri  [128, 128] upper-tri (incl diag) causal mask for [k, q] layout
    Output: part [B, D, S] f32 Wo partial, d-major."""
    nc = bacc.Bacc("TRN2", target_bir_lowering=False, debug=False,
                   num_devices=N_CORES)
    h16_d = nc.dram_tensor("h16", [128, T], F16, kind="ExternalInput")
    h32_d = nc.dram_tensor("h32", [128, T], F32R, kind="ExternalInput")
    eqs_d = nc.dram_tensor("eqs", [128, B, 128], F16, kind="ExternalInput")
    eks_d = nc.dram_tensor("eks", [128, B, 128], F16, kind="ExternalInput")
    evs_d = nc.dram_tensor("evs", [128, B, 128], F32R, kind="ExternalInput")
    woT_d = nc.dram_tensor("woT", [128, D], F32R, kind="ExternalInput")
    tri_d = nc.dram_tensor("tri", [128, 128], F32R, kind="ExternalInput")
    part_d = nc.dram_tensor("part", [B, D, S], F32, kind="ExternalOutput")

    with tile.TileContext(nc) as tc:
        with (
            tc.tile_pool(name="big", bufs=1) as big,
            tc.tile_pool(name="work", bufs=3) as work,
            tc.tile_pool(name="small", bufs=6) as small,
            tc.tile_pool(name="psA", bufs=4, space="PSUM") as psA,
            tc.tile_pool(name="psB", bufs=3, space="PSUM") as psB,
            tc.tile_pool(name="outp", bufs=4) as outp,
        ):
            h16 = big.tile([128, T], F16, tag="h16")
            nc.sync.dma_start(h16[:], h16_d.ap())
            eq = big.tile([128, B, 128], F16, tag="eq")
            nc.scalar.dma_start(eq[:], eqs_d.ap())
            ek = big.tile([128, B, 128], F16, tag="ek")
            nc.sync.dma_start(ek[:], eks_d.ap())
            h32 = big.tile([128, T], F32R, tag="h32")
            nc.sync.dma_start(h32[:], h32_d.ap())
            ev = big.tile([128, B, 128], F32R, tag="ev")
            nc.scalar.dma_start(ev[:], evs_d.ap())
            tri = big.tile([128, 128], F32R, tag="tri")
            nc.gpsimd.dma_start(tri[:], tri_d.ap())
            wo = big.tile([128, D], F32R, tag="wo")
            nc.scalar.dma_start(wo[:], woT_d.ap())
            ones64 = big.tile([1, 64], F32R, tag="ones64")
            nc.gpsimd.memset(ones64[:].bitcast(F32), 1.0)

            qt = big.tile([128, B, S], F16, tag="qt")
            kt = big.tile([128, B, S], F16, tag="kt")
            vt = big.tile([128, B, 4, 130], F32R, tag="vt")
            for b in range(B):
                for st in range(4):
                    nc.gpsimd.memset(vt[:, b, st, 64:65].bitcast(F32), 1.0)
                    nc.gpsimd.memset(vt[:, b, st, 129:130].bitcast(F32), 1.0)

            # QKV projections
            for b in range(B):
                qp = psA.tile([128, S], F32, tag="mm")
                nc.tensor.matmul(qp[:], eq[:, b, :], h16[:, b * S:(b + 1) * S])
                nc.vector.tensor_copy(qt[:, b, :], qp[:])
                kp = psA.tile([128, S], F32, tag="mm")
                nc.tensor.matmul(kp[:], ek[:, b, :], h16[:, b * S:(b + 1) * S])
                nc.vector.tensor_copy(kt[:, b, :], kp[:])
                for st in range(4):
                    vp = psB.tile([128, S], F32, tag="vv", name="vp")[:, :128]
                    nc.tensor.matmul(
                        vp[:], h32[:, b * S + st * 128:b * S + (st + 1) * 128],
                        ev[:, b, :])
                    nc.vector.tensor_copy(vt[:, b, st, 0:64], vp[:, 0:64])
                    nc.vector.tensor_copy(vt[:, b, st, 65:129], vp[:, 64:128])

            # attention + Wo, per batch element (Wo(b=0) overlaps attn(b=1))
            att = big.tile([128, B, S], F32R, tag="att")
            oti = 0
            for b in range(B):
                for hh in range(2):
                    p0 = 64 * hh
                    et = work.tile([128, 4, S], F32R, tag="et")
                    for k in range(4):
                        q0 = 128 * k
                        sp = psA.tile([128, S], F32, tag="mm")
                        nc.tensor.matmul(
                            sp[:, q0:S],
                            kt[p0:p0 + 64, b, q0:q0 + 128],
                            qt[p0:p0 + 64, b, q0:S])
                        nc.scalar.activation(et[:, k, q0:S], sp[:, q0:S],
                                             EXPF, scale=float(1.0 / np.sqrt(DH)))
                        nc.gpsimd.tensor_mul(et[:, k, q0:q0 + 128],
                                             et[:, k, q0:q0 + 128], tri[:])
                    # out rows 0:64 = O^T [dh, q]; row 64 = Z[q] via ones col
                    op = psB.tile([128, S], F32, tag="vv")
                    for k in range(4):
                        nc.tensor.matmul(
                            op[0:65, 128 * k:S],
                            vt[:, b, k, 65 * hh:65 * (hh + 1)],
                            et[:, k, 128 * k:S],
                            start=(k == 0), stop=(k == 3))
                    zr = small.tile([1, S], F32, tag="zr")
                    nc.vector.reciprocal(zr[:], op[64:65, :])
                    zrr = small.tile([1, S], F32R, tag="zrr")
                    nc.gpsimd.tensor_copy(zrr[:], zr[:])
                    zbp = psA.tile([128, S], F32, tag="mm", name="zbp")[:64, :]
                    nc.tensor.matmul(zbp[:], ones64[:], zrr[:])
                    zb = small.tile([64, S], F32, tag="zb")
                    nc.scalar.activation(zb[:], zbp[:], COPYF)
                    nc.vector.tensor_mul(att[p0:p0 + 64, b, :], op[0:64, :],
                                         zb[:])
                # Wo partial for this batch element
                for mt in range(DT):
                    wp = psA.tile([128, S], F32, tag="mm")
                    nc.tensor.matmul(wp[:], wo[:, mt * 128:(mt + 1) * 128],
                                     att[:, b, :])
                    ot = outp.tile([128, S], F32, tag="ot")
                    if oti % 2:
                        nc.vector.tensor_copy(ot[:], wp[:])
                    else:
                        nc.scalar.activation(ot[:], wp[:], COPYF)
                    nc.sync.dma_start(part_d.ap()[b, mt * 128:(mt + 1) * 128, :],
                                      ot[:])
                    oti += 1
    nc.compile()
    return nc


def _build_C(wv16: bool):
    """Memory module, token-sharded (128 tokens per core). Inputs:
      QT  [128(R), 128]  host-computed Q^T slice, pre-scaled by 1/sqrt(R)
      kKT [128(R), NK]   knowledge_K.T
      kv  [128, NT, D]   knowledge_V, nk-major tiles (f32r or f16)
    Output: mo [128, D] f32, rows = this core's tokens."""
    ET = F16 if wv16 else F32R
    nc = bacc.Bacc("TRN2", target_bir_lowering=False, debug=False,
                   num_devices=N_CORES)
    qt_d = nc.dram_tensor("QT", [128, 128], F32, kind="ExternalInput")
    kk_d = nc.dram_tensor("kKT", [128, NK], F32, kind="ExternalInput")
    kv_d = nc.dram_tensor("kv", [128, NT, D], ET, kind="ExternalInput")
    mo_d = nc.dram_tensor("mo", [128, D], F32, kind="ExternalOutput")

    with tile.TileContext(nc) as tc:
        with (
            tc.tile_pool(name="big", bufs=1) as big,
            tc.tile_pool(name="work", bufs=1) as work,
            tc.tile_pool(name="psA", bufs=3, space="PSUM") as psA,
            tc.tile_pool(name="psB", bufs=3, space="PSUM") as psB,
            tc.tile_pool(name="psF", bufs=2, space="PSUM") as psF,
        ):
            q = big.tile([128, 128], F32, tag="q")
            nc.sync.dma_start(q[:], qt_d.ap())
            kk = big.tile([128, NK], F32, tag="kk")
            nc.scalar.dma_start(kk[:], kk_d.ap())
            kv = big.tile([128, NT, D], ET, tag="kv")
            nc.sync.dma_start(kv[:], kv_d.ap())
            idn = big.tile([128, 128], F32, tag="idn")
            make_identity(nc, idn[:])

            # scores token-major [tok, NK], strict f32
            s = work.tile([128, NK], F32, tag="s")
            for c2 in range(2):
                sp = psA.tile([128, 512], F32, tag="mm")
                nc.tensor.matmul(sp[:], q[:], kk[:, c2 * 512:(c2 + 1) * 512])
                nc.scalar.activation(s[:, c2 * 512:(c2 + 1) * 512], sp[:], COPYF)

            # PE warm-up fillers: keep the tensor engine's p-state ramp
            # alive through the DVE top-16 phase so WV runs at peak clock.
            fills = []
            for f in range(8):
                fp = psF.tile([128, 512], F32, tag="fill", name="fill")
                nc.tensor.matmul(fp[:], q[:], kk[:, 0:512])
                fills.append(fp)

            # exact top-16 threshold: 3 DVE passes; tau = 16th value, s >= tau
            m8a = work.tile([128, 8], F32, tag="m8a")
            m8b = work.tile([128, 8], F32, tag="m8b")
            s2 = work.tile([128, NK], F32, tag="s2")
            nc.vector.max(m8a[:], s[:])
            nc.vector.match_replace(s2[:], m8a[:], s[:], NEG)
            nc.vector.max(m8b[:], s2[:])
            negm = work.tile([128, 1], F32, tag="negm")
            nc.gpsimd.tensor_scalar_mul(negm[:], m8a[:, 0:1], -1.0)

            # masked exp + Z (fused), token-major
            et = work.tile([128, NK], F32, tag="et")
            nc.scalar.activation(et[:], s[:], EXPF, bias=negm[:])
            msk = work.tile([128, NK], F32, tag="msk")
            nc.vector.tensor_scalar(msk[:], s[:], m8b[:, 7:8], scalar2=None,
                                    op0=mybir.AluOpType.is_ge)
            etm = work.tile([128, NK], F32, tag="etm")
            nc.vector.tensor_mul(etm[:], et[:], msk[:])
            zs = work.tile([128, 1], F32, tag="zs")
            nc.vector.reduce_sum(zs[:], etm[:], axis=mybir.AxisListType.X)
            zr = work.tile([128, 1], F32, tag="zrec")
            nc.vector.reciprocal(zr[:], zs[:])

            # transpose to nk-major, then WV
            etT = work.tile([128, NT, 128], ET, tag="etT")
            for nt in range(NT):
                tp = psB.tile([128, 128], F32, tag="tp")
                nc.tensor.transpose(tp[:], etm[:, nt * 128:(nt + 1) * 128],
                                    idn[:])
                if nt % 2:
                    nc.vector.tensor_copy(etT[:, nt, :], tp[:])
                else:
                    nc.scalar.activation(etT[:, nt, :], tp[:], COPYF)
            out = work.tile([128, D], F32, tag="out")
            for c2 in range(2):
                wp = psA.tile([128, 512], F32, tag="mm")
                for nt in range(NT):
                    nc.tensor.matmul(wp[:], etT[:, nt, :],
                                     kv[:, nt, c2 * 512:(c2 + 1) * 512],
                                     start=(nt == 0), stop=(nt == NT - 1))
                nc.vector.tensor_scalar_mul(out[:, c2 * 512:(c2 + 1) * 512],
                                            wp[:], zr[:])
            nc.sync.dma_start(mo_d.ap(), out[:])
    nc.compile()
    return nc


def _build_D():
    """lm_head, vocab-sharded, fp16. Inputs: xfT [D, T]; hwT [D, VSL].
    Output: lo [T, VSL] f16."""
    nc = bacc.Bacc("TRN2", target_bir_lowering=False, debug=False,
                   num_devices=N_CORES)
    xf_d = nc.dram_tensor("xfT", [D, T], F16, kind="ExternalInput")
    hw_d = nc.dram_tensor("hwT", [D, VSL], F16, kind="ExternalInput")
    lo_d = nc.dram_tensor("lo", [T, VSL], F16, kind="ExternalOutput")


    with tile.TileContext(nc) as tc:
        with (
            tc.tile_pool(name="xfp", bufs=8) as xfp,
            tc.tile_pool(name="wpool", bufs=16) as wpool,
            tc.tile_pool(name="opool", bufs=6) as opool,
            tc.tile_pool(name="ps", bufs=8, space="PSUM") as ps,
        ):
            # per-dt xf tiles so accumulation can start as chunks land
            xf = []
            for dt in range(DT):
                t = xfp.tile([128, T], F16, tag=f"xf{dt}")
                eng = nc.sync if dt % 2 else nc.scalar
                eng.dma_start(t[:], xf_d.ap()[dt * 128:(dt + 1) * 128, :])
                xf.append(t)
            qi = 0
            for vc in range(NVC):
                hw = []
                for dt in range(DT):
                    t = wpool.tile([128, VC], F16, tag="hw")
                    eng = nc.sync if qi % 2 else nc.scalar
                    eng.dma_start(
                        t[:], hw_d.ap()[dt * 128:(dt + 1) * 128,
                                        vc * VC:(vc + 1) * VC])
                    qi += 1
                    hw.append(t)
                for tt in range(DT):
                    pp = ps.tile([128, VC], F32, tag="pp")
                    for dt in range(DT):
                        nc.tensor.matmul(pp[:],
                                         xf[dt][:, tt * 128:(tt + 1) * 128],
                                         hw[dt][:],
                                         start=(dt == 0), stop=(dt == DT - 1))
                    ot = opool.tile([128, VC], F16, tag="ot")
                    if tt % 2:
                        nc.vector.tensor_copy(ot[:], pp[:])
                    else:
                        nc.scalar.activation(ot[:], pp[:], COPYF)
                    nc.gpsimd.dma_start(
                        lo_d.ap()[tt * 128:(tt + 1) * 128,
                                  vc * VC:(vc + 1) * VC], ot[:])
    nc.compile()
    return nc


_PROGS = {}


def _prog(name):
    if name not in _PROGS:
        _PROGS[name] = {
            "A": _build_A,
            "C0": lambda: _build_C(False),
            "C1": lambda: _build_C(True),
            "D": _build_D,
        }[name]()
    return _PROGS[name]


# ---------------------------------------------------------------- host-side math


def _ln(x, w, b):
    m = x.mean(-1, keepdims=True, dtype=np.float32)
    v = ((x - m) ** 2).mean(-1, keepdims=True, dtype=np.float32)
    return ((x - m) / np.sqrt(v + np.float32(1e-5)) * w + b).astype(np.float32)


def _softmax(x, axis=-1):
    m = x.max(axis=axis, keepdims=True)
    e = np.exp(x - m)
    return e / e.sum(axis=axis, keepdims=True)


def _nw(xn, A, Bm, Wimp, Wr):
    """SSM scan + routing -> neuron weights [B, NC] (host, fp32)."""
    u = xn @ Bm                       # [B,S,SD]
    h = np.zeros((xn.shape[0], A.shape[0]), np.float32)
    for t in range(xn.shape[1]):
        h = h @ A + u[:, t]
    h_proj = h @ Wimp.T               # [B, D]
    imp = _softmax(np.einsum('bsd,bd->bs', xn, h_proj), axis=-1)
    pref = _softmax(xn @ Wr.T, axis=-1)
    nw = np.einsum('bs,bsn->bn', imp, pref)
    return (nw / (nw.sum(-1, keepdims=True) + np.float32(1e-8))).astype(np.float32)


_run_ncores = list(range(N_CORES))


def _run(name, in_maps):
    res = run_bass_kernel_spmd(_prog(name), in_maps, core_ids=_run_ncores)
    return res.results


_CONV_CACHE = {}


def _conv(key, arr_id, fn):
    ent = _CONV_CACHE.get(key)
    if ent is None or ent[0] != arr_id:
        _CONV_CACHE[key] = ent = (arr_id, fn())
    return ent[1]


def kernel(**inputs) -> np.ndarray:
    inp = {k: np.asarray(v) for k, v in inputs.items()}
    ids = inp['input_ids'].astype(np.int64)
    comp_f = inp['compress_neurons'].reshape(NC, -1).astype(np.float32)
    tri = np.triu(np.ones((128, 128), np.float32))
    kKT = np.ascontiguousarray(inp['knowledge_K'].T, dtype=np.float32)
    kv32 = _conv('kv32', id(inp['knowledge_V']), lambda: np.ascontiguousarray(
        inp['knowledge_V'].astype(np.float32).reshape(NT, 128, D)
        .transpose(1, 0, 2)))
    kv16 = _conv('kv16', id(inp['knowledge_V']),
                 lambda: kv32.astype(np.float16))

    x = (inp['tok_emb'][ids] + inp['pos_emb'][None, :ids.shape[1]]).astype(np.float32)

    for l in range(L):
        # ---- circuit (device program A, head-sharded) ----
        xn = _ln(x, inp['ln1_w'][l], inp['ln1_b'][l])
        nw = _nw(xn, inp['a_A'][l], inp['a_B'][l], inp['a_imp'][l], inp['a_router'][l])
        sc = (nw @ comp_f).reshape(B, D, R)
        eq = (nw @ inp['eQ'][l].reshape(NC, -1).astype(np.float32)).reshape(B, R, D)
        ek = (nw @ inp['eK'][l].reshape(NC, -1).astype(np.float32)).reshape(B, R, D)
        ev = (nw @ inp['eV'][l].reshape(NC, -1).astype(np.float32)).reshape(B, R, D)
        h = np.einsum('bsd,bdr->bsr', xn, sc)           # [B,S,R]
        hT32 = np.ascontiguousarray(
            np.concatenate([h[b].T for b in range(B)], axis=1), dtype=np.float32)
        hT16 = hT32.astype(np.float16)
        woT = np.ascontiguousarray(inp['o_w'][l].T, dtype=np.float32)
        in_maps = []
        for c in range(N_CORES):
            sl = slice(128 * c, 128 * (c + 1))
            in_maps.append({
                "h16": hT16, "h32": hT32,
                "eqs": np.ascontiguousarray(
                    eq[:, :, sl].transpose(1, 0, 2), dtype=np.float16),
                "eks": np.ascontiguousarray(
                    ek[:, :, sl].transpose(1, 0, 2), dtype=np.float16),
                "evs": np.ascontiguousarray(
                    ev[:, :, sl].transpose(1, 0, 2), dtype=np.float32),
                "woT": np.ascontiguousarray(woT[sl, :]),
                "tri": tri,
            })
        res = _run("A", in_maps)
        circT = res[0]["part"].astype(np.float32)
        for c in range(1, N_CORES):
            circT = circT + res[c]["part"]
        x = x + circT.transpose(0, 2, 1)

        # ---- memory (device program C0/C1, token-sharded) ----
        xn = _ln(x, inp['ln2_w'][l], inp['ln2_b'][l])
        nw = _nw(xn, inp['m_A'][l], inp['m_B'][l], inp['m_imp'][l], inp['m_router'][l])
        sc = (nw @ comp_f).reshape(B, D, R) * np.float32(1.0 / np.sqrt(R))
        Q = np.einsum('bsd,bdr->bsr', xn, sc)           # [B,S,R] pre-scaled
        in_maps = []
        for c in range(N_CORES):
            bc, s0 = c // 4, 128 * (c % 4)
            in_maps.append({
                "QT": np.ascontiguousarray(Q[bc, s0:s0 + 128, :].T),
                "kKT": kKT,
                "kv": kv32 if l == 0 else kv16,
            })
        res = _run("C0" if l == 0 else "C1", in_maps)
        mo = np.empty((B, S, D), np.float32)
        for c in range(N_CORES):
            bc, s0 = c // 4, 128 * (c % 4)
            mo[bc, s0:s0 + 128] = res[c]["mo"]
        x = x + mo

    # ---- lm_head (device program D, vocab-sharded, fp16) ----
    xf = _ln(x, inp['lnf_w'], inp['lnf_b'])
    xfT = np.ascontiguousarray(
        np.concatenate([xf[b].T for b in range(B)], axis=1), dtype=np.float16)
    hwT = _conv('hwT', id(inp['head_w']), lambda: np.ascontiguousarray(
        inp['head_w'].T, dtype=np.float16))
    in_maps = [{"xfT": xfT,
                "hwT": np.ascontiguousarray(hwT[:, VSL * c:VSL * (c + 1)])}
               for c in range(N_CORES)]
    res = _run("D", in_maps)
    logits = np.concatenate([res[c]["lo"].astype(np.float32)
                             for c in range(N_CORES)], axis=1)
    return logits.reshape(B, S, V)


# revision 16
# speedup vs baseline: 1.1557x; 1.1557x over previous
"""Trainium2 Bass kernel for nn_DAWN_41549513621652.

Strategy (8 NeuronCores, single chip, no cross-core collectives):
  Dense matmul work (attention, Wo, memory WV, lm_head) runs on device;
  sequential/tiny glue (layernorm, the 512-step SSM scan, routing softmax,
  the rank-128 compress projections h=xn@sc / Q=xn@sc, and the neuron-pool
  contractions nw@{comp,EQ,EK,EV}) runs on host between launches; host also
  sums the 8 per-core Wo partials.

  5 device launches per call:
    A (x2): circuit module, head-sharded — core c owns heads {2c, 2c+1} for
            both batch elements. fp16 is used ONLY for the Q/K score path
            (attention scores are ~1e-4, softmax er-uniform, insensitive);
            the V path / attention output / Wo stay f32r and partials are
            written fp32: the downstream memory module's top-16 selection
            has 16/17-gaps down to 7e-9 and amplifies any upstream drift
            >1e-6 into >2e-2 logit errors (measured).  The softmax
            denominator Z is produced by an extra ones-column appended to V
            in the same accumulating matmul as the attention output.
    C (x2): memory module, token-sharded — core c owns 128 tokens.  Exact
            top-16 in 3 DVE passes (max8 / match_replace / max8, then
            select s >= 16th value), masked exp + Z fused in one
            tensor_tensor_reduce, PE-transpose to nk-major, dense WV
            matmul.  Scores strictly fp32 both layers; layer 1's WV/kV are
            fp16 (they feed only the lm_head, no selection downstream).
    D (x1): lm_head, vocab-sharded — core c owns a 4000-wide slice of the
            32000 vocab, entirely fp16 (halves the 36MB DMA, keeps the PE
            at its ~107us roofline).
"""

import numpy as np

import concourse.bass as bass
import concourse.bacc as bacc
import concourse.mybir as mybir
import concourse.tile as tile
from concourse.bass_utils import run_bass_kernel_spmd
from concourse.masks import make_identity

F32 = mybir.dt.float32
F32R = mybir.dt.float32r
F16 = mybir.dt.float16

# model dims (hardcoded per problem spec)
L, D, H, R, NC, NK, KK, SD, V, B, S = 2, 1024, 16, 128, 64, 1024, 16, 64, 32000, 2, 512
DH = D // H          # 64
T = B * S            # 1024
N_CORES = 8
VSL = V // N_CORES   # 4000 per-core vocab slice
VC = 500             # vocab chunk (psum tile width)
NVC = VSL // VC      # 8
DT = D // 128        # 8 d-tiles
NT = NK // 128       # 8 knowledge tiles
NEG = -1e30
EXPF = mybir.ActivationFunctionType.Exp
COPYF = mybir.ActivationFunctionType.Copy


# ---------------------------------------------------------------- device programs


def _build_A():
    """Circuit module, head-sharded. Per-core inputs:
      h16/h32 [128(R), T]  compressed tokens (host: xn@sc), R-major
      eqs/eks [128(R), B, 128] f16, evs f32  expansion slices (2 heads)
      woT  [128, D]  o_w.T rows for this core's d_in slice
      tri  [128, 128] upper-tri (incl diag) causal mask for [k, q] layout
    Output: part [B, D, S] f32 Wo partial, d-major."""
    nc = bacc.Bacc("TRN2", target_bir_lowering=False, debug=False,
                   num_devices=N_CORES)
    h16_d = nc.dram_tensor("h16", [128, T], F16, kind="ExternalInput")
    h32_d = nc.dram_tensor("h32", [128, T], F32R, kind="ExternalInput")
    eqs_d = nc.dram_tensor("eqs", [128, B, 128], F16, kind="ExternalInput")
    eks_d = nc.dram_tensor("eks", [128, B, 128], F16, kind="ExternalInput")
    evs_d = nc.dram_tensor("evs", [128, B, 128], F32R, kind="ExternalInput")
    woT_d = nc.dram_tensor("woT", [128, D], F32R, kind="ExternalInput")
    tri_d = nc.dram_tensor("tri", [128, 128], F32R, kind="ExternalInput")
    part_d = nc.dram_tensor("part", [B, D, S], F32, kind="ExternalOutput")

    with tile.TileContext(nc) as tc:
        with (
            tc.tile_pool(name="big", bufs=1) as big,
            tc.tile_pool(name="work", bufs=3) as work,
            tc.tile_pool(name="small", bufs=6) as small,
            tc.tile_pool(name="psA", bufs=4, space="PSUM") as psA,
            tc.tile_pool(name="psB", bufs=3, space="PSUM") as psB,
            tc.tile_pool(name="outp", bufs=4) as outp,
        ):
            h16 = big.tile([128, T], F16, tag="h16")
            nc.sync.dma_start(h16[:], h16_d.ap())
            eq = big.tile([128, B, 128], F16, tag="eq")
            nc.scalar.dma_start(eq[:], eqs_d.ap())
            ek = big.tile([128, B, 128], F16, tag="ek")
            nc.sync.dma_start(ek[:], eks_d.ap())
            h32 = big.tile([128, T], F32R, tag="h32")
            nc.sync.dma_start(h32[:], h32_d.ap())
            ev = big.tile([128, B, 128], F32R, tag="ev")
            nc.scalar.dma_start(ev[:], evs_d.ap())
            tri = big.tile([128, 128], F32R, tag="tri")
            nc.gpsimd.dma_start(tri[:], tri_d.ap())
            wo = big.tile([128, D], F32R, tag="wo")
            nc.scalar.dma_start(wo[:], woT_d.ap())
            ones64 = big.tile([1, 64], F32R, tag="ones64")
            nc.gpsimd.memset(ones64[:].bitcast(F32), 1.0)

            qt = big.tile([128, B, S], F16, tag="qt")
            kt = big.tile([128, B, S], F16, tag="kt")
            vt = big.tile([128, B, 4, 130], F32R, tag="vt")
            for b in range(B):
                for st in range(4):
                    nc.gpsimd.memset(vt[:, b, st, 64:65].bitcast(F32), 1.0)
                    nc.gpsimd.memset(vt[:, b, st, 129:130].bitcast(F32), 1.0)

            # QKV projections
            for b in range(B):
                qp = psA.tile([128, S], F32, tag="mm")
                nc.tensor.matmul(qp[:], eq[:, b, :], h16[:, b * S:(b + 1) * S])
                nc.vector.tensor_copy(qt[:, b, :], qp[:])
                kp = psA.tile([128, S], F32, tag="mm")
                nc.tensor.matmul(kp[:], ek[:, b, :], h16[:, b * S:(b + 1) * S])
                nc.vector.tensor_copy(kt[:, b, :], kp[:])
                for st in range(4):
                    vp = psB.tile([128, S], F32, tag="vv", name="vp")[:, :128]
                    nc.tensor.matmul(
                        vp[:], h32[:, b * S + st * 128:b * S + (st + 1) * 128],
                        ev[:, b, :])
                    nc.vector.tensor_copy(vt[:, b, st, 0:64], vp[:, 0:64])
                    nc.vector.tensor_copy(vt[:, b, st, 65:129], vp[:, 64:128])

            # attention: stage-interleaved across the 4 (b, head) units so
            # the PE never idles (keeps the p-state ramp at peak).  Order:
            # QK(u0) QK(u1) AV(u0) AV(u1) QK(u2) QK(u3) Wo(b0) AV(u2) AV(u3) Wo(b1)
            att = big.tile([128, B, S], F32R, tag="att")
            units = [(b, hh) for b in range(B) for hh in range(2)]
            ets = {}
            oti = 0

            def qk_stage(u):
                b, hh = units[u]
                p0 = 64 * hh
                et = work.tile([128, 4, S], F32R, tag="et", name=f"et{u}")
                ets[u] = et
                for k in range(4):
                    q0 = 128 * k
                    sp = psA.tile([128, S], F32, tag="mm", name=f"sp{u}_{k}")
                    nc.tensor.matmul(
                        sp[:, q0:S],
                        kt[p0:p0 + 64, b, q0:q0 + 128],
                        qt[p0:p0 + 64, b, q0:S])
                    nc.scalar.activation(et[:, k, q0:S], sp[:, q0:S],
                                         EXPF, scale=float(1.0 / np.sqrt(DH)))
                    nc.gpsimd.tensor_mul(et[:, k, q0:q0 + 128],
                                         et[:, k, q0:q0 + 128], tri[:])

            def av_stage(u):
                b, hh = units[u]
                p0 = 64 * hh
                et = ets[u]
                op = psB.tile([128, S], F32, tag="vv", name=f"op{u}")
                for k in range(4):
                    nc.tensor.matmul(
                        op[0:65, 128 * k:S],
                        vt[:, b, k, 65 * hh:65 * (hh + 1)],
                        et[:, k, 128 * k:S],
                        start=(k == 0), stop=(k == 3))
                zr = small.tile([1, S], F32, tag="zr", name=f"zr{u}")
                nc.vector.reciprocal(zr[:], op[64:65, :])
                zrr = small.tile([1, S], F32R, tag="zrr", name=f"zrr{u}")
                nc.gpsimd.tensor_copy(zrr[:], zr[:])
                zbp = psA.tile([128, S], F32, tag="mm", name=f"zbp{u}")[:64, :]
                nc.tensor.matmul(zbp[:], ones64[:], zrr[:])
                zb = small.tile([64, S], F32, tag="zb", name=f"zb{u}")
                nc.scalar.activation(zb[:], zbp[:], COPYF)
                nc.vector.tensor_mul(att[p0:p0 + 64, b, :], op[0:64, :], zb[:])

            def wo_stage(b):
                nonlocal oti
                for mt in range(DT):
                    wp = psA.tile([128, S], F32, tag="mm", name=f"wp{b}_{mt}")
                    nc.tensor.matmul(wp[:], wo[:, mt * 128:(mt + 1) * 128],
                                     att[:, b, :])
                    ot = outp.tile([128, S], F32, tag="ot", name=f"ot{b}_{mt}")
                    if oti % 2:
                        nc.vector.tensor_copy(ot[:], wp[:])
                    else:
                        nc.scalar.activation(ot[:], wp[:], COPYF)
                    nc.sync.dma_start(part_d.ap()[b, mt * 128:(mt + 1) * 128, :],
                                      ot[:])
                    oti += 1

            qk_stage(0)
            qk_stage(1)
            av_stage(0)
            av_stage(1)
            qk_stage(2)
            qk_stage(3)
            wo_stage(0)
            av_stage(2)
            av_stage(3)
            wo_stage(1)
    nc.compile()
    return nc


def _build_C(wv16: bool):
    """Memory module, token-sharded (128 tokens per core). Inputs:
      QT  [128(R), 128]  host-computed Q^T slice, pre-scaled by 1/sqrt(R)
      kKT [128(R), NK]   knowledge_K.T
      kv  [128, NT, D]   knowledge_V, nk-major tiles (f32r or f16)
    Output: mo [128, D] f32, rows = this core's tokens."""
    ET = F16 if wv16 else F32R
    nc = bacc.Bacc("TRN2", target_bir_lowering=False, debug=False,
                   num_devices=N_CORES)
    qt_d = nc.dram_tensor("QT", [128, 128], F32, kind="ExternalInput")
    kk_d = nc.dram_tensor("kKT", [128, NK], F32, kind="ExternalInput")
    kv_d = nc.dram_tensor("kv", [128, NT, D], ET, kind="ExternalInput")
    mo_d = nc.dram_tensor("mo", [128, D], F32, kind="ExternalOutput")

    with tile.TileContext(nc) as tc:
        with (
            tc.tile_pool(name="big", bufs=1) as big,
            tc.tile_pool(name="work", bufs=1) as work,
            tc.tile_pool(name="psA", bufs=3, space="PSUM") as psA,
            tc.tile_pool(name="psB", bufs=3, space="PSUM") as psB,
        ):
            q = big.tile([128, 128], F32, tag="q")
            nc.sync.dma_start(q[:], qt_d.ap())
            kk = big.tile([128, NK], F32, tag="kk")
            nc.scalar.dma_start(kk[:], kk_d.ap())
            kv = big.tile([128, NT, D], ET, tag="kv")
            nc.sync.dma_start(kv[:], kv_d.ap())
            idn = big.tile([128, 128], F32, tag="idn")
            make_identity(nc, idn[:])

            # scores token-major [tok, NK], strict f32
            s = work.tile([128, NK], F32, tag="s")
            for c2 in range(2):
                sp = psA.tile([128, 512], F32, tag="mm")
                nc.tensor.matmul(sp[:], q[:], kk[:, c2 * 512:(c2 + 1) * 512])
                nc.scalar.activation(s[:, c2 * 512:(c2 + 1) * 512], sp[:], COPYF)

            # exact top-16 threshold: 3 DVE passes; tau = 16th value, s >= tau
            m8a = work.tile([128, 8], F32, tag="m8a")
            m8b = work.tile([128, 8], F32, tag="m8b")
            s2 = work.tile([128, NK], F32, tag="s2")
            nc.vector.max(m8a[:], s[:])
            nc.vector.match_replace(s2[:], m8a[:], s[:], NEG)
            nc.vector.max(m8b[:], s2[:])
            negm = work.tile([128, 1], F32, tag="negm")
            nc.gpsimd.tensor_scalar_mul(negm[:], m8a[:, 0:1], -1.0)

            # masked exp + Z (fused), token-major
            et = work.tile([128, NK], F32, tag="et")
            nc.scalar.activation(et[:], s[:], EXPF, bias=negm[:])
            msk = work.tile([128, NK], F32, tag="msk")
            nc.vector.tensor_scalar(msk[:], s[:], m8b[:, 7:8], scalar2=None,
                                    op0=mybir.AluOpType.is_ge)
            etm = work.tile([128, NK], F32, tag="etm")
            nc.vector.tensor_mul(etm[:], et[:], msk[:])
            zs = work.tile([128, 1], F32, tag="zs")
            nc.vector.reduce_sum(zs[:], etm[:], axis=mybir.AxisListType.X)
            zr = work.tile([128, 1], F32, tag="zrec")
            nc.vector.reciprocal(zr[:], zs[:])

            # transpose to nk-major, then WV
            etT = work.tile([128, NT, 128], ET, tag="etT")
            for nt in range(NT):
                tp = psB.tile([128, 128], F32, tag="tp")
                nc.tensor.transpose(tp[:], etm[:, nt * 128:(nt + 1) * 128],
                                    idn[:])
                if nt % 2:
                    nc.vector.tensor_copy(etT[:, nt, :], tp[:])
                else:
                    nc.scalar.activation(etT[:, nt, :], tp[:], COPYF)
            out = work.tile([128, D], F32, tag="out")
            for c2 in range(2):
                wp = psA.tile([128, 512], F32, tag="mm")
                for nt in range(NT):
                    nc.tensor.matmul(wp[:], etT[:, nt, :],
                                     kv[:, nt, c2 * 512:(c2 + 1) * 512],
                                     start=(nt == 0), stop=(nt == NT - 1))
                nc.vector.tensor_scalar_mul(out[:, c2 * 512:(c2 + 1) * 512],
                                            wp[:], zr[:])
                nc.sync.dma_start(mo_d.ap()[:, c2 * 512:(c2 + 1) * 512],
                                  out[:, c2 * 512:(c2 + 1) * 512])
    nc.compile()
    return nc


def _build_D():
    """lm_head, vocab-sharded, fp16. Inputs: xfT [D, T]; hwT [D, VSL].
    Output: lo [T, VSL] f16."""
    nc = bacc.Bacc("TRN2", target_bir_lowering=False, debug=False,
                   num_devices=N_CORES)
    xf_d = nc.dram_tensor("xfT", [D, T], F16, kind="ExternalInput")
    hw_d = nc.dram_tensor("hwT", [D, VSL], F16, kind="ExternalInput")
    lo_d = nc.dram_tensor("lo", [T, VSL], F16, kind="ExternalOutput")


    with tile.TileContext(nc) as tc:
        with (
            tc.tile_pool(name="xfp", bufs=8) as xfp,
            tc.tile_pool(name="wpool", bufs=16) as wpool,
            tc.tile_pool(name="opool", bufs=6) as opool,
            tc.tile_pool(name="ps", bufs=8, space="PSUM") as ps,
        ):
            # per-dt xf tiles, paired with vc0's hw chunks so the first
            # accumulations can start as soon as each (xf[dt], hw[0][dt]) lands
            xf = []
            hw0 = []
            for dt in range(DT):
                t = xfp.tile([128, T], F16, tag=f"xf{dt}")
                nc.sync.dma_start(t[:], xf_d.ap()[dt * 128:(dt + 1) * 128, :])
                xf.append(t)
                w = wpool.tile([128, VC], F16, tag="hw", name=f"hw0_{dt}")
                nc.sync.dma_start(
                    w[:], hw_d.ap()[dt * 128:(dt + 1) * 128, 0:VC])
                hw0.append(w)

            def emit_out(vc, tt, pp):
                ot = opool.tile([128, VC], F16, tag="ot", name=f"ot{vc}_{tt}")
                nc.vector.tensor_copy(ot[:], pp[:])
                nc.scalar.dma_start(
                    lo_d.ap()[tt * 128:(tt + 1) * 128,
                              vc * VC:(vc + 1) * VC], ot[:])

            # vc 0: dt-major so the PE starts with the first arriving chunk
            pps = []
            for tt in range(DT):
                pps.append(ps.tile([128, VC], F32, tag="pp", name=f"pp0_{tt}"))
            for dt in range(DT):
                for tt in range(DT):
                    nc.tensor.matmul(pps[tt][:],
                                     xf[dt][:, tt * 128:(tt + 1) * 128],
                                     hw0[dt][:],
                                     start=(dt == 0), stop=(dt == DT - 1))
            for tt in range(DT):
                emit_out(0, tt, pps[tt])

            # vc 1..7: tt-major (weights prefetched a chunk ahead)
            for vc in range(1, NVC):
                hw = []
                for dt in range(DT):
                    t = wpool.tile([128, VC], F16, tag="hw", name=f"hw{vc}_{dt}")
                    nc.sync.dma_start(
                        t[:], hw_d.ap()[dt * 128:(dt + 1) * 128,
                                        vc * VC:(vc + 1) * VC])
                    hw.append(t)
                for tt in range(DT):
                    pp = ps.tile([128, VC], F32, tag="pp", name=f"pp{vc}_{tt}")
                    for dt in range(DT):
                        nc.tensor.matmul(pp[:],
                                         xf[dt][:, tt * 128:(tt + 1) * 128],
                                         hw[dt][:],
                                         start=(dt == 0), stop=(dt == DT - 1))
                    emit_out(vc, tt, pp)
    nc.compile()
    return nc


_PROGS = {}


def _prog(name):
    if name not in _PROGS:
        _PROGS[name] = {
            "A": _build_A,
            "C0": lambda: _build_C(False),
            "C1": lambda: _build_C(True),
            "D": _build_D,
        }[name]()
    return _PROGS[name]


# ---------------------------------------------------------------- host-side math


def _ln(x, w, b):
    m = x.mean(-1, keepdims=True, dtype=np.float32)
    v = ((x - m) ** 2).mean(-1, keepdims=True, dtype=np.float32)
    return ((x - m) / np.sqrt(v + np.float32(1e-5)) * w + b).astype(np.float32)


def _softmax(x, axis=-1):
    m = x.max(axis=axis, keepdims=True)
    e = np.exp(x - m)
    return e / e.sum(axis=axis, keepdims=True)


def _nw(xn, A, Bm, Wimp, Wr):
    """SSM scan + routing -> neuron weights [B, NC] (host, fp32)."""
    u = xn @ Bm                       # [B,S,SD]
    h = np.zeros((xn.shape[0], A.shape[0]), np.float32)
    for t in range(xn.shape[1]):
        h = h @ A + u[:, t]
    h_proj = h @ Wimp.T               # [B, D]
    imp = _softmax(np.einsum('bsd,bd->bs', xn, h_proj), axis=-1)
    pref = _softmax(xn @ Wr.T, axis=-1)
    nw = np.einsum('bs,bsn->bn', imp, pref)
    return (nw / (nw.sum(-1, keepdims=True) + np.float32(1e-8))).astype(np.float32)


_run_ncores = list(range(N_CORES))


def _run(name, in_maps):
    res = run_bass_kernel_spmd(_prog(name), in_maps, core_ids=_run_ncores)
    return res.results


_CONV_CACHE = {}


def _conv(key, arr_id, fn):
    ent = _CONV_CACHE.get(key)
    if ent is None or ent[0] != arr_id:
        _CONV_CACHE[key] = ent = (arr_id, fn())
    return ent[1]


def kernel(**inputs) -> np.ndarray:
    inp = {k: np.asarray(v) for k, v in inputs.items()}
    ids = inp['input_ids'].astype(np.int64)
    comp_f = inp['compress_neurons'].reshape(NC, -1).astype(np.float32)
    tri = np.triu(np.ones((128, 128), np.float32))
    kKT = np.ascontiguousarray(inp['knowledge_K'].T, dtype=np.float32)
    kv32 = _conv('kv32', id(inp['knowledge_V']), lambda: np.ascontiguousarray(
        inp['knowledge_V'].astype(np.float32).reshape(NT, 128, D)
        .transpose(1, 0, 2)))
    kv16 = _conv('kv16', id(inp['knowledge_V']),
                 lambda: kv32.astype(np.float16))

    x = (inp['tok_emb'][ids] + inp['pos_emb'][None, :ids.shape[1]]).astype(np.float32)

    for l in range(L):
        # ---- circuit (device program A, head-sharded) ----
        xn = _ln(x, inp['ln1_w'][l], inp['ln1_b'][l])
        nw = _nw(xn, inp['a_A'][l], inp['a_B'][l], inp['a_imp'][l], inp['a_router'][l])
        sc = (nw @ comp_f).reshape(B, D, R)
        eq = (nw @ inp['eQ'][l].reshape(NC, -1).astype(np.float32)).reshape(B, R, D)
        ek = (nw @ inp['eK'][l].reshape(NC, -1).astype(np.float32)).reshape(B, R, D)
        ev = (nw @ inp['eV'][l].reshape(NC, -1).astype(np.float32)).reshape(B, R, D)
        h = np.einsum('bsd,bdr->bsr', xn, sc)           # [B,S,R]
        hT32 = np.ascontiguousarray(
            np.concatenate([h[b].T for b in range(B)], axis=1), dtype=np.float32)
        hT16 = hT32.astype(np.float16)
        woT = np.ascontiguousarray(inp['o_w'][l].T, dtype=np.float32)
        in_maps = []
        for c in range(N_CORES):
            sl = slice(128 * c, 128 * (c + 1))
            in_maps.append({
                "h16": hT16, "h32": hT32,
                "eqs": np.ascontiguousarray(
                    eq[:, :, sl].transpose(1, 0, 2), dtype=np.float16),
                "eks": np.ascontiguousarray(
                    ek[:, :, sl].transpose(1, 0, 2), dtype=np.float16),
                "evs": np.ascontiguousarray(
                    ev[:, :, sl].transpose(1, 0, 2), dtype=np.float32),
                "woT": np.ascontiguousarray(woT[sl, :]),
                "tri": tri,
            })
        res = _run("A", in_maps)
        circT = res[0]["part"].astype(np.float32)
        for c in range(1, N_CORES):
            circT = circT + res[c]["part"]
        x = x + circT.transpose(0, 2, 1)

        # ---- memory (device program C0/C1, token-sharded) ----
        xn = _ln(x, inp['ln2_w'][l], inp['ln2_b'][l])
        nw = _nw(xn, inp['m_A'][l], inp['m_B'][l], inp['m_imp'][l], inp['m_router'][l])
        sc = (nw @ comp_f).reshape(B, D, R) * np.float32(1.0 / np.sqrt(R))
        Q = np.einsum('bsd,bdr->bsr', xn, sc)           # [B,S,R] pre-scaled
        in_maps = []
        for c in range(N_CORES):
            bc, s0 = c // 4, 128 * (c % 4)
            in_maps.append({
                "QT": np.ascontiguousarray(Q[bc, s0:s0 + 128, :].T),
                "kKT": kKT,
                "kv": kv32 if l == 0 else kv16,
            })
        res = _run("C0" if l == 0 else "C1", in_maps)
        mo = np.empty((B, S, D), np.float32)
        for c in range(N_CORES):
            bc, s0 = c // 4, 128 * (c % 4)
            mo[bc, s0:s0 + 128] = res[c]["mo"]
        x = x + mo

    # ---- lm_head (device program D, vocab-sharded, fp16) ----
    xf = _ln(x, inp['lnf_w'], inp['lnf_b'])
    xfT = np.ascontiguousarray(
        np.concatenate([xf[b].T for b in range(B)], axis=1), dtype=np.float16)
    hwT = _conv('hwT', id(inp['head_w']), lambda: np.ascontiguousarray(
        inp['head_w'].T, dtype=np.float16))
    in_maps = [{"xfT": xfT,
                "hwT": np.ascontiguousarray(hwT[:, VSL * c:VSL * (c + 1)])}
               for c in range(N_CORES)]
    res = _run("D", in_maps)
    logits = np.concatenate([res[c]["lo"].astype(np.float32)
                             for c in range(N_CORES)], axis=1)
    return logits.reshape(B, S, V)
